# revision 1
# baseline (speedup 1.0000x reference)
"""Trainium2 Bass kernel for multi-head cross-attention.

Reference computation (fp32):
  q = x @ Wq; k = ctx @ Wk; v = ctx @ Wv              (per batch)
  sim = einsum('bihd,bjhd->bhij', q, k) * 1/sqrt(64)
  out = softmax(sim) @ v ; out = out @ Wo + bo

Shapes: x (4, 2048, 1024), context (4, 2048, 768), HEADS=8, DIM_HEAD=64.

Sharding: 8 cores = (batch b = core//2) x (query half = core%2). Each core
computes the full attention for its 1024 query rows across all 8 heads with
replicated weights; outputs concatenate — no cross-core reduction.

On-core dataflow. Matmul operands are bf16 (1 cycle/row on the PE; fp32r
measures ~2 cycles/row on TRN2 HW) with fp32 PSUM accumulation:
  - x^T and ctx^T are prepared host-side (feature dim on partitions),
    pre-cast to bf16 on host along with the weights.
  - q^T[c,i], k^T[c,j]  via lhsT=W, rhs=x^T/ctx^T   (feature-major outputs)
  - v[j,c]              via lhsT=ctx^T, rhs=Wv       (context-major output),
    stored per head with an extra ones column: [v_h | 1] (65 cols/head)
  - per head: S^T[j,i] = k_h @ q_h^T  (lhsT=k^T slice, rhs=q^T)
    exp on ACT with scale=1/8 folded in, bf16 out; PV matmul lhsT=[v_h|1]
    accumulates O'[0:64]=unnormalized attn out (transposed) and
    O'[64]=softmax denominator, in one fp32 PSUM accumulation group.
  - normalize (entirely off the PE so its instruction queue never stalls —
    a PE-visible wait on the reciprocal re-throttles the HAM clock gate):
    O' is copied to SBUF immediately (frees the PSUM slot for the next
    head's PV), recip(denom) on DVE, a lane-shift DMA moves it to
    partition 0, gpsimd partition_broadcast replicates it over 64 lanes,
    and a DVE mult normalizes. Odd heads are lane-shifted into the stacked
    O^T layout via a SBUF->SBUF DMA (DVE is lane-locked).
  - final: F = O^T.T @ Wo + ones^T @ bo (bias via K=1 matmul into the same
    PSUM accumulation group).

Input DMAs are split per 128-row tile so the first projection matmuls
start as soon as their operands land rather than after the full tensor.
"""

import ml_dtypes
import numpy as np

import concourse.bass as bass
import concourse.tile as tile
from concourse import bacc, mybir
from concourse.bass_utils import run_bass_kernel_spmd

F32 = mybir.dt.float32
BF16 = mybir.dt.bfloat16

B = 4
NQ_FULL = 2048
NQ = 1024  # local query rows per core
NC = 2048
DQ = 1024
DC = 768
H = 8
DH = 64
INNER = H * DH  # 512
SCALE = DH ** -0.5

AT = DQ // 128   # 8  k-tiles of the q-projection contraction
BT = DC // 128   # 6  k-tiles of the k/v-projection contraction
CT = INNER // 128  # 4 feature tiles of q^T/k^T/o^T
IB = NQ // 128   # 8  query-row blocks
JB = NC // 128   # 16 context-row blocks

_CACHE = {}


def _build_program():
    nc = bacc.Bacc(
        "TRN2",
        target_bir_lowering=False,
        debug=False,
        enable_asserts=False,
    )

    xT = nc.dram_tensor("xT", [DQ, NQ], BF16, kind="ExternalInput").ap()
    ctxT = nc.dram_tensor("ctxT", [DC, NC], BF16, kind="ExternalInput").ap()
    wq = nc.dram_tensor("Wq", [DQ, INNER], BF16, kind="ExternalInput").ap()
    wk = nc.dram_tensor("Wk", [DC, INNER], BF16, kind="ExternalInput").ap()
    wv = nc.dram_tensor("Wv", [DC, INNER], BF16, kind="ExternalInput").ap()
    wo = nc.dram_tensor("Wo", [INNER, DQ], BF16, kind="ExternalInput").ap()
    bo = nc.dram_tensor("bo", [DQ], BF16, kind="ExternalInput").ap()
    out = nc.dram_tensor("out", [NQ, DQ], F32, kind="ExternalOutput").ap()

    with tile.TileContext(nc) as tc:
        with nc.allow_low_precision(reason="bf16 matmul operands"):
            _emit(nc, tc, xT, ctxT, wq, wk, wv, wo, bo, out)

    nc.compile()
    return nc


def _emit(nc, tc, xT, ctxT, wq, wk, wv, wo, bo, out):
    from contextlib import ExitStack

    with ExitStack() as ctx:
        const = ctx.enter_context(tc.tile_pool(name="const", bufs=1))
        persist = ctx.enter_context(tc.tile_pool(name="persist", bufs=1))
        expp = ctx.enter_context(tc.tile_pool(name="expp", bufs=4))
        opool = ctx.enter_context(tc.tile_pool(name="opool", bufs=3))
        rpool = ctx.enter_context(tc.tile_pool(name="rpool", bufs=2))
        otmp = ctx.enter_context(tc.tile_pool(name="otmp", bufs=2))
        outp = ctx.enter_context(tc.tile_pool(name="outp", bufs=2))
        ps_a = ctx.enter_context(tc.tile_pool(name="ps_a", bufs=2, space="PSUM"))
        ps_o = ctx.enter_context(tc.tile_pool(name="ps_o", bufs=2, space="PSUM"))

        # --- constants ---
        bo_sb = const.tile([1, DQ], BF16)
        nc.sync.dma_start(out=bo_sb, in_=bo.unsqueeze(0))
        onesF = const.tile([128, 128], F32)
        nc.vector.memset(onesF, 1.0)
        ones1 = const.tile([1, 128], BF16)  # bias-matmul lhsT
        nc.vector.tensor_copy(ones1, onesF[0:1, :])

        # --- persistent feature-major activations ---
        qT_sb = persist.tile([128, CT, NQ], BF16)
        kT_sb = persist.tile([128, CT, NC], BF16)
        v_sb = persist.tile([128, JB, H * 65], BF16)  # [v_h | 1] per head
        oT_sb = persist.tile([128, CT, NQ], BF16)
        wo_sb = persist.tile([128, CT, DQ], BF16)
        # Wo prefetch: queued first so it lands long before the projection ends.
        wor = wo.rearrange("(t p) e -> p t e", p=128)
        for t in range(CT):
            nc.sync.dma_start(out=wo_sb[:, t, :], in_=wor[:, t, :])

        v4 = v_sb.rearrange("p j (h e) -> p j h e", e=65)
        for jb in range(JB):
            nc.vector.tensor_copy(v4[:, jb, :, 64:65], onesF[:, 0:H].unsqueeze(-1))

        # --- phase A: q^T = (x @ Wq)^T via lhsT=Wq, rhs=x^T ---
        with tc.tile_pool(name="phA", bufs=1) as phA:
            xT_sb = phA.tile([128, AT, NQ], BF16)
            wq_sb = phA.tile([128, AT, INNER], BF16)
            xTr = xT.rearrange("(t p) i -> p t i", p=128)
            wqr = wq.rearrange("(t p) c -> p t c", p=128)
            for a in range(AT):
                nc.sync.dma_start(out=wq_sb[:, a, :], in_=wqr[:, a, :])
                nc.sync.dma_start(out=xT_sb[:, a, :], in_=xTr[:, a, :])
            for t in range(CT):
                ps = ps_a.tile([128, NQ], F32, tag="pa")
                for a in range(AT):
                    for ch in range(2):
                        nc.tensor.matmul(
                            ps[:, ch * 512:(ch + 1) * 512],
                            lhsT=wq_sb[:, a, t * 128:(t + 1) * 128],
                            rhs=xT_sb[:, a, ch * 512:(ch + 1) * 512],
                            start=(a == 0),
                            stop=(a == AT - 1),
                        )
                nc.vector.tensor_copy(qT_sb[:, t, :], ps)

        # --- phase B: k^T and v from streamed ctx^T quarters ---
        with tc.tile_pool(name="phBw", bufs=1) as phBw:
            wk_sb = phBw.tile([128, BT, INNER], BF16)
            wv_sb = phBw.tile([128, BT, INNER], BF16)
            wkr = wk.rearrange("(t p) c -> p t c", p=128)
            wvr = wv.rearrange("(t p) c -> p t c", p=128)
            for b in range(BT):
                nc.sync.dma_start(out=wk_sb[:, b, :], in_=wkr[:, b, :])
                nc.sync.dma_start(out=wv_sb[:, b, :], in_=wvr[:, b, :])
            ctxTr = ctxT.rearrange("(t p) j -> p t j", p=128)
            with tc.tile_pool(name="phBx", bufs=2) as phBx:
                for jq in range(4):
                    cx = phBx.tile([128, BT, 512], BF16, tag="cx")
                    for b in range(BT):
                        nc.sync.dma_start(
                            out=cx[:, b, :],
                            in_=ctxTr[:, b, jq * 512:(jq + 1) * 512],
                        )
                    for t in range(CT):
                        ps = ps_a.tile([128, NQ], F32, tag="pa")
                        for b in range(BT):
                            nc.tensor.matmul(
                                ps[:, 0:512],
                                lhsT=wk_sb[:, b, t * 128:(t + 1) * 128],
                                rhs=cx[:, b, :],
                                start=(b == 0),
                                stop=(b == BT - 1),
                            )
                        nc.vector.tensor_copy(
                            kT_sb[:, t, jq * 512:(jq + 1) * 512], ps[:, 0:512]
                        )
                    for q in range(4):
                        jb = jq * 4 + q
                        ps = ps_a.tile([128, NQ], F32, tag="pa")
                        for b in range(BT):
                            nc.tensor.matmul(
                                ps[:, 0:512],
                                lhsT=cx[:, b, q * 128:(q + 1) * 128],
                                rhs=wv_sb[:, b, :],
                                start=(b == 0),
                                stop=(b == BT - 1),
                            )
                        nc.vector.tensor_copy(
                            v4[:, jb, :, 0:64],
                            ps[:, 0:512].rearrange("p (h d) -> p h d", d=DH),
                        )

        # --- attention per head ---
        for h in range(H):
            t, po = h // 2, 64 * (h % 2)
            qTh = qT_sb[po:po + 64, t, :]
            kTh = kT_sb[po:po + 64, t, :]
            ops = ps_o.tile([128, NQ], F32, tag="po")  # rows 0-63 O'; row 64 denom
            for jb in range(JB):
                sps = ps_a.tile([128, NQ], F32, tag="pa")
                for ch in range(2):
                    nc.tensor.matmul(
                        sps[:, ch * 512:(ch + 1) * 512],
                        lhsT=kTh[:, jb * 128:(jb + 1) * 128],
                        rhs=qTh[:, ch * 512:(ch + 1) * 512],
                        start=True,
                        stop=True,
                    )
                es = expp.tile([128, NQ], BF16, tag="es")
                nc.scalar.activation(
                    es, sps, mybir.ActivationFunctionType.Exp, scale=SCALE
                )
                for ch in range(2):
                    nc.tensor.matmul(
                        ops[0:65, ch * 512:(ch + 1) * 512],
                        lhsT=v4[:, jb, h, :],
                        rhs=es[:, ch * 512:(ch + 1) * 512],
                        start=(jb == 0),
                        stop=(jb == JB - 1),
                    )
            # Evacuate O' to SBUF immediately: frees the ps_o slot so the next
            # head's PV can start while this head normalizes off the PE.
            osb = opool.tile([65, NQ], F32, tag="osb")
            nc.vector.tensor_copy(osb, ops[0:65, :])
            # normalize: O^T = O'[0:64] * (1/denom) broadcast over partitions
            rt = rpool.tile([65, NQ], F32, tag="rt")
            nc.vector.reciprocal(rt[64:65, :], osb[64:65, :])
            r0 = rpool.tile([1, NQ], F32, tag="r0")
            nc.sync.dma_start(out=r0, in_=rt[64:65, :])  # lane 64 -> lane 0
            rbx = rpool.tile([64, NQ], F32, tag="rbx")
            nc.gpsimd.partition_broadcast(rbx, r0)
            if h % 2 == 0:
                nc.vector.tensor_mul(oT_sb[0:64, t, :], osb[0:64, :], rbx)
            else:
                ot = otmp.tile([64, NQ], BF16, tag="ot")
                nc.vector.tensor_mul(ot, osb[0:64, :], rbx)
                nc.sync.dma_start(out=oT_sb[64:128, t, :], in_=ot)

        # --- output projection: F = O^T.T @ Wo + bias ---
        for ib in range(IB):
            fp = ps_a.tile([128, NQ], F32, tag="pa")
            for ch in range(2):
                for t in range(CT):
                    nc.tensor.matmul(
                        fp[:, ch * 512:(ch + 1) * 512],
                        lhsT=oT_sb[:, t, ib * 128:(ib + 1) * 128],
                        rhs=wo_sb[:, t, ch * 512:(ch + 1) * 512],
                        start=(t == 0),
                        stop=False,
                    )
                nc.tensor.matmul(
                    fp[:, ch * 512:(ch + 1) * 512],
                    lhsT=ones1,
                    rhs=bo_sb[0:1, ch * 512:(ch + 1) * 512],
                    start=False,
                    stop=True,
                )
            ost = outp.tile([128, DQ], F32)
            nc.vector.tensor_copy(ost, fp)
            nc.sync.dma_start(out=out[ib * 128:(ib + 1) * 128, :], in_=ost)


def get_program():
    if "nc" not in _CACHE:
        _CACHE["nc"] = _build_program()
    return _CACHE["nc"]


def make_in_maps(x, context, Wq, Wk, Wv, Wo, bo):
    bf = ml_dtypes.bfloat16
    in_maps = []
    wq_b = np.asarray(Wq).astype(bf)
    wk_b = np.asarray(Wk).astype(bf)
    wv_b = np.asarray(Wv).astype(bf)
    wo_b = np.asarray(Wo).astype(bf)
    bo_b = np.asarray(bo).astype(bf)
    for c in range(8):
        b, half = c // 2, c % 2
        in_maps.append({
            "xT": np.ascontiguousarray(
                x[b, half * NQ:(half + 1) * NQ, :].T
            ).astype(bf),
            "ctxT": np.ascontiguousarray(context[b].T).astype(bf),
            "Wq": wq_b,
            "Wk": wk_b,
            "Wv": wv_b,
            "Wo": wo_b,
            "bo": bo_b,
        })
    return in_maps


def kernel(x, context, Wq, Wk, Wv, Wo, bo):
    nc = get_program()
    in_maps = make_in_maps(x, context, Wq, Wk, Wv, Wo, bo)
    res = run_bass_kernel_spmd(nc, in_maps, list(range(8)))
    out = np.empty((B, NQ_FULL, DQ), np.float32)
    for c in range(8):
        b, half = c // 2, c % 2
        out[b, half * NQ:(half + 1) * NQ, :] = res.results[c]["out"]
    return out



# revision 17
# speedup vs baseline: 1.1546x; 1.1546x over previous
"""Trainium2 Bass kernel for multi-head cross-attention.

Reference computation (fp32):
  q = x @ Wq; k = ctx @ Wk; v = ctx @ Wv              (per batch)
  sim = einsum('bihd,bjhd->bhij', q, k) * 1/sqrt(64)
  out = softmax(sim) @ v ; out = out @ Wo + bo

Shapes: x (4, 2048, 1024), context (4, 2048, 768), HEADS=8, DIM_HEAD=64.

Sharding: 8 cores = (batch b = core//2) x (head-group g = core%2, 4 heads
each). Each core computes q/k/v for its 4 heads only (no redundant K/V
projection work), full attention over all 2048 queries, and a PARTIAL
output projection F_g = O_g @ Wo[g-rows]. The host gather sums the two
partial F's per batch and adds the bias — no cross-core communication.

On-core dataflow. Matmul operands are bf16 with fp32 PSUM accumulation:
  - x^T and ctx^T prepared host-side (feature dim on partitions), pre-cast
    to bf16 along with the sliced weights.
  - q^T[c,i], k^T[c,j]  via lhsT=W, rhs=x^T/ctx^T   (feature-major outputs)
  - v[j,c]              via lhsT=ctx^T, rhs=Wv       (context-major output),
    stored per head with an extra ones column: [v_h | 1] (65 cols/head)
  - attention runs in 8 passes over (head h, query half ih), each pass
    1024 query rows: S^T[j,i] = k_h @ q_h^T (lhsT=k^T slice), exp on ACT
    with scale=1/8 folded in, bf16 out; PV matmul lhsT=[v_h|1] accumulates
    O'[0:64] (transposed unnormalized out) and O'[64] (softmax denominator)
    in one fp32 PSUM accumulation group.
  - normalize off the PE: O' evacuated to SBUF immediately (frees the PSUM
    slot for the next pass), reciprocal_approx_fast on DVE (~5x faster than
    reciprocal), gpsimd partition_broadcast replicates 1/denom over 64
    lanes directly from partition 64, DVE mult normalizes. Odd heads are
    lane-shifted into the stacked O^T layout via a SBUF->SBUF DMA.
    Pass order ends on an even head so the last (serial) normalize chain
    is the cheap direct-write variant.
  - final: F_partial = O^T.T @ Wo_slice per 128-row block; f32 out.
    Bias is NOT added on-device (host adds it once after summing partials).

Input DMAs are split per 128-row tile and ordered so the first projection
matmuls start as soon as their operands land; Wo is prefetched after the
k/v weights so it never delays the projection phase.
"""

import ml_dtypes
import numpy as np

import concourse.bass as bass
import concourse.tile as tile
from concourse import bacc, mybir
from concourse.bass_utils import run_bass_kernel_spmd

F32 = mybir.dt.float32
BF16 = mybir.dt.bfloat16

B = 4
NQ = 2048        # query rows per core (full)
NC = 2048
DQ = 1024
DC = 768
H = 8            # total heads
HL = 4           # heads per core
DH = 64
INNER = H * DH           # 512
INNER_L = HL * DH        # 256 local inner dim
SCALE = DH ** -0.5

AT = DQ // 128     # 8  k-tiles of the q-projection contraction
BT = DC // 128     # 6  k-tiles of the k/v-projection contraction
CT = INNER_L // 128  # 2 feature tiles of q^T/k^T/o^T
IB = NQ // 128     # 16 query-row blocks
JB = NC // 128     # 16 context-row blocks
IH = 2             # query halves per attention pass
NQH = NQ // IH     # 1024

_CACHE = {}


def _build_program():
    nc = bacc.Bacc(
        "TRN2",
        target_bir_lowering=False,
        debug=False,
        enable_asserts=False,
    )

    # Weights arrive host-packed partition-major ([128, tiles*cols]) so each
    # DMA row is a single >=3KB contiguous descriptor (512B rows run at less
    # than half the per-descriptor throughput on the dynamic DMA engines).
    xT = nc.dram_tensor("xT", [DQ, NQ], BF16, kind="ExternalInput").ap()
    ctxT = nc.dram_tensor("ctxT", [DC, NC], BF16, kind="ExternalInput").ap()
    wq = nc.dram_tensor("Wq", [128, AT * INNER_L], BF16, kind="ExternalInput").ap()
    wk = nc.dram_tensor("Wk", [128, BT * INNER_L], BF16, kind="ExternalInput").ap()
    wv = nc.dram_tensor("Wv", [128, BT * INNER_L], BF16, kind="ExternalInput").ap()
    wo = nc.dram_tensor("Wo", [128, CT * DQ], BF16, kind="ExternalInput").ap()
    # Row-block pairs packed side by side: DRAM row = 4KB (one descriptor).
    out = nc.dram_tensor("out", [NQ // 2, 2 * DQ], BF16, kind="ExternalOutput").ap()

    with tile.TileContext(nc) as tc:
        with nc.allow_low_precision(reason="bf16 matmul operands"):
            _emit(nc, tc, xT, ctxT, wq, wk, wv, wo, out)

    nc.compile()
    return nc


def _emit(nc, tc, xT, ctxT, wq, wk, wv, wo, out):
    from contextlib import ExitStack

    with ExitStack() as ctx:
        const = ctx.enter_context(tc.tile_pool(name="const", bufs=1))
        persist = ctx.enter_context(tc.tile_pool(name="persist", bufs=1))
        expp = ctx.enter_context(tc.tile_pool(name="expp", bufs=4))
        opool = ctx.enter_context(tc.tile_pool(name="opool", bufs=3))
        rpool = ctx.enter_context(tc.tile_pool(name="rpool", bufs=2))
        otmp = ctx.enter_context(tc.tile_pool(name="otmp", bufs=2))
        outp = ctx.enter_context(tc.tile_pool(name="outp", bufs=2))
        ps_a = ctx.enter_context(tc.tile_pool(name="ps_a", bufs=2, space="PSUM"))
        ps_o = ctx.enter_context(tc.tile_pool(name="ps_o", bufs=2, space="PSUM"))

        onesF = const.tile([128, 8], F32)
        nc.vector.memset(onesF, 1.0)

        # --- persistent feature-major activations ---
        qT_sb = persist.tile([128, CT, NQ], BF16)
        kT_sb = persist.tile([128, CT, NC], BF16)
        v_sb = persist.tile([128, JB, HL * 65], BF16)  # [v_h | 1] per head
        oT_sb = persist.tile([128, CT, NQ], BF16)
        wo_sb = persist.tile([128, CT, DQ], BF16)

        v4 = v_sb.rearrange("p j (h e) -> p j h e", e=65)
        for jb in range(JB):
            nc.vector.tensor_copy(v4[:, jb, :, 64:65], onesF[:, 0:HL].unsqueeze(-1))

        # --- phase A: q^T = (x @ Wq)^T via lhsT=Wq, rhs=x^T ---
        with tc.tile_pool(name="phA", bufs=1) as phA:
            xT_sb = phA.tile([128, AT, NQ], BF16)
            wq_sb = phA.tile([128, AT, INNER_L], BF16)
            xTr = xT.rearrange("(t p) i -> p t i", p=128)
            nc.sync.dma_start(out=wq_sb, in_=wq.rearrange("p (t c) -> p t c", t=AT))
            for a in range(AT):
                nc.sync.dma_start(out=xT_sb[:, a, :], in_=xTr[:, a, :])
            for t in range(CT):
                for ih in range(IH):
                    ps = ps_a.tile([128, NQH], F32, tag="pa")
                    for a in range(AT):
                        for ch in range(2):
                            nc.tensor.matmul(
                                ps[:, ch * 512:(ch + 1) * 512],
                                lhsT=wq_sb[:, a, t * 128:(t + 1) * 128],
                                rhs=xT_sb[:, a, ih * NQH + ch * 512:
                                          ih * NQH + (ch + 1) * 512],
                                start=(a == 0),
                                stop=(a == AT - 1),
                            )
                    nc.vector.tensor_copy(
                        qT_sb[:, t, ih * NQH:(ih + 1) * NQH], ps
                    )

        # --- phase B: k^T and v from streamed ctx^T quarters ---
        with tc.tile_pool(name="phBw", bufs=1) as phBw:
            wk_sb = phBw.tile([128, BT, INNER_L], BF16)
            wv_sb = phBw.tile([128, BT, INNER_L], BF16)
            nc.sync.dma_start(out=wk_sb, in_=wk.rearrange("p (t c) -> p t c", t=BT))
            nc.sync.dma_start(out=wv_sb, in_=wv.rearrange("p (t c) -> p t c", t=BT))
            ctxTr = ctxT.rearrange("(t p) j -> p t j", p=128)
            with tc.tile_pool(name="phBx", bufs=2) as phBx:
                for jq in range(4):
                    cx = phBx.tile([128, BT, 512], BF16, tag="cx")
                    for b in range(BT):
                        nc.sync.dma_start(
                            out=cx[:, b, :],
                            in_=ctxTr[:, b, jq * 512:(jq + 1) * 512],
                        )
                    if jq == 1:
                        # Wo prefetch: after the first ctx quarter is queued so
                        # it never delays phase B, but lands well before the
                        # output projection.
                        nc.sync.dma_start(
                            out=wo_sb,
                            in_=wo.rearrange("p (t e) -> p t e", t=CT),
                        )
                    for t in range(CT):
                        ps = ps_a.tile([128, NQH], F32, tag="pa")
                        for b in range(BT):
                            nc.tensor.matmul(
                                ps[:, 0:512],
                                lhsT=wk_sb[:, b, t * 128:(t + 1) * 128],
                                rhs=cx[:, b, :],
                                start=(b == 0),
                                stop=(b == BT - 1),
                            )
                        nc.vector.tensor_copy(
                            kT_sb[:, t, jq * 512:(jq + 1) * 512], ps[:, 0:512]
                        )
                    for q in range(4):
                        jb = jq * 4 + q
                        ps = ps_a.tile([128, NQH], F32, tag="pa")
                        for b in range(BT):
                            nc.tensor.matmul(
                                ps[:, 0:INNER_L],
                                lhsT=cx[:, b, q * 128:(q + 1) * 128],
                                rhs=wv_sb[:, b, :],
                                start=(b == 0),
                                stop=(b == BT - 1),
                            )
                        nc.vector.tensor_copy(
                            v4[:, jb, :, 0:64],
                            ps[:, 0:INNER_L].rearrange("p (h d) -> p h d", d=DH),
                        )

        # --- attention: 8 passes over (head, query half) ---
        # Order ends on an even head (direct oT write) so the tail normalize
        # chain is cheap; final-projection matmuls fill the remaining gap.
        passes = [(1, 0), (1, 1), (3, 0), (3, 1), (0, 0), (0, 1), (2, 1), (2, 0)]
        for h, ih in passes:
            t, po = h // 2, 64 * (h % 2)
            i0 = ih * NQH
            qTh = qT_sb[po:po + 64, t, i0:i0 + NQH]
            kTh = kT_sb[po:po + 64, t, :]
            ops = ps_o.tile([128, NQH], F32, tag="po")  # rows 0-63 O'; row 64 denom
            for jb in range(JB):
                sps = ps_a.tile([128, NQH], F32, tag="pa")
                for ch in range(2):
                    nc.tensor.matmul(
                        sps[:, ch * 512:(ch + 1) * 512],
                        lhsT=kTh[:, jb * 128:(jb + 1) * 128],
                        rhs=qTh[:, ch * 512:(ch + 1) * 512],
                        start=True,
                        stop=True,
                    )
                es = expp.tile([128, NQH], BF16, tag="es")
                nc.scalar.activation(
                    es, sps, mybir.ActivationFunctionType.Exp, scale=SCALE
                )
                for ch in range(2):
                    nc.tensor.matmul(
                        ops[0:65, ch * 512:(ch + 1) * 512],
                        lhsT=v4[:, jb, h, :],
                        rhs=es[:, ch * 512:(ch + 1) * 512],
                        start=(jb == 0),
                        stop=(jb == JB - 1),
                    )
            # Evacuate O' to SBUF immediately: frees the ps_o slot so the next
            # pass's PV can start while this one normalizes off the PE.
            osb = opool.tile([65, NQH], F32, tag="osb")
            nc.vector.tensor_copy(osb, ops[0:65, :])
            rt = rpool.tile([65, NQH], F32, tag="rt")
            nc.vector.reciprocal(rt[64:65, :], osb[64:65, :])
            # partition_broadcast on HW reads physical partition 0 regardless
            # of the AP base partition (CoreSim honors it) — lane-shift first.
            r0 = rpool.tile([1, NQH], F32, tag="r0")
            nc.sync.dma_start(out=r0, in_=rt[64:65, :])
            rbx = rpool.tile([64, NQH], F32, tag="rbx")
            nc.gpsimd.partition_broadcast(rbx, r0)
            if h % 2 == 0:
                nc.vector.tensor_mul(oT_sb[0:64, t, i0:i0 + NQH], osb[0:64, :], rbx)
            else:
                ot = otmp.tile([64, NQH], BF16, tag="ot")
                nc.vector.tensor_mul(ot, osb[0:64, :], rbx)
                nc.sync.dma_start(out=oT_sb[64:128, t, i0:i0 + NQH], in_=ot)

        # --- output projection: F_partial = O^T.T @ Wo_slice (no bias) ---
        # Row-block pairs from the second query half first: their oT operands
        # are ready one attention pass earlier, so ~10µs of matmuls execute
        # under the last pass's normalize chain instead of stalling behind it.
        # Each pair's two [128, 1024] results share one SBUF tile so the
        # output DMA rows are 4KB (2KB bf16 rows halve DMA throughput).
        for j in (4, 5, 6, 7, 0, 1, 2, 3):
            ost = outp.tile([128, 2, DQ], BF16)
            for half in range(2):
                ib = 2 * j + half
                fp = ps_a.tile([128, DQ], F32, tag="pa")
                for ch in range(2):
                    for t in range(CT):
                        nc.tensor.matmul(
                            fp[:, ch * 512:(ch + 1) * 512],
                            lhsT=oT_sb[:, t, ib * 128:(ib + 1) * 128],
                            rhs=wo_sb[:, t, ch * 512:(ch + 1) * 512],
                            start=(t == 0),
                            stop=(t == CT - 1),
                        )
                nc.vector.tensor_copy(ost[:, half, :], fp)
            nc.sync.dma_start(out=out[j * 128:(j + 1) * 128, :],
                              in_=ost.rearrange("p a e -> p (a e)"))


def get_program():
    if "nc" not in _CACHE:
        _CACHE["nc"] = _build_program()
    return _CACHE["nc"]


def _pack(w):
    """[T*128, C] -> [128, T*C] partition-major (row p = concat_t w[t*128+p])."""
    t = w.shape[0] // 128
    return np.ascontiguousarray(
        w.reshape(t, 128, w.shape[1]).transpose(1, 0, 2).reshape(128, -1)
    )


def make_in_maps(x, context, Wq, Wk, Wv, Wo, bo):
    bf = ml_dtypes.bfloat16
    in_maps = []
    wq_b = np.asarray(Wq).astype(bf)
    wk_b = np.asarray(Wk).astype(bf)
    wv_b = np.asarray(Wv).astype(bf)
    wo_b = np.asarray(Wo).astype(bf)
    for c in range(8):
        b, g = c // 2, c % 2
        cs = g * INNER_L
        in_maps.append({
            "xT": np.ascontiguousarray(x[b].T).astype(bf),
            "ctxT": np.ascontiguousarray(context[b].T).astype(bf),
            "Wq": _pack(wq_b[:, cs:cs + INNER_L]),
            "Wk": _pack(wk_b[:, cs:cs + INNER_L]),
            "Wv": _pack(wv_b[:, cs:cs + INNER_L]),
            "Wo": _pack(wo_b[cs:cs + INNER_L, :]),
        })
    return in_maps


def kernel(x, context, Wq, Wk, Wv, Wo, bo):
    nc = get_program()
    in_maps = make_in_maps(x, context, Wq, Wk, Wv, Wo, bo)
    res = run_bass_kernel_spmd(nc, in_maps, list(range(8)))
    bo32 = np.asarray(bo, np.float32)
    out = np.empty((B, NQ, DQ), np.float32)
    for b in range(B):
        out[b] = (unpack_out(res.results[2 * b]["out"])
                  + unpack_out(res.results[2 * b + 1]["out"]) + bo32)
    return out


def unpack_out(packed):
    """[1024, 2048] pair-packed bf16 -> [2048, 1024] f32 partial F."""
    return (packed.astype(np.float32)
            .reshape(8, 128, 2, DQ).transpose(0, 2, 1, 3).reshape(NQ, DQ))
